# revision 3
# baseline (speedup 1.0000x reference)
"""MinGRU Trainium2 kernel (v3).

Problem: B=8, T=4096, D=512, H=512 MinGRU:
    k' = x @ Wz^T + bz;  z = sigmoid(k')
    w' = x @ Wh^T + bh;  g(w') = relu(w') + min(sigmoid(w'), 0.5)
    h_t = (1 - z_t) * h_{t-1} + z_t * g_t,   h_{-1} = g(h_0)

Key identities:
    g(v) = max(sigmoid(v), v + 0.5)          (exact)
    h    = a * h + z*g  with a = 1 - z       (linear-space scan, fp32 state)

Sharding: data-parallel over batch, one batch row per NeuronCore (8 cores).

Engine assignment per (chunk, hb) tile of [128, TC<=1024]:
    PE:      kp = x8 @ Wz8 (fp8e4 DoubleRow, 2x rate), wp = xb @ Wh (bf16)
    ScalarE: z = sigmoid(kp + bz); s = sigmoid(wp + bh); t = wp + bh + 0.5
    DVE:     a = 1 - z (tensor_scalar, 4x); g = max(t, s) (tensor_tensor, 2x)
             bn = z * g (tensor_tensor, 2x); scan (2 cyc/col, the floor)

fp8 only on the z-path: its error is strongly attenuated by the sigmoid and
by h being a convex combination; the h-path (g) stays bf16. Measured numpy
end-to-end rel err ~4.6e-3 (vs 2e-2 budget).
"""

import os

import numpy as np

import concourse.bass as bass
import concourse.mybir as mybir
import concourse.tile as tile
from concourse import bacc
from concourse.bass_utils import run_bass_kernel_spmd

B, T, D, H = 8, 4096, 512, 512
P = 128
HB = H // P
CHUNKS = [512, 1024, 1024, 1024, 512]
assert sum(CHUNKS) == T

F32 = mybir.dt.float32
BF16 = mybir.dt.bfloat16
FP8 = mybir.dt.float8e4
EW = BF16

BN_GP = bool(int(os.environ.get("MINGRU_BN_GP", "0")))  # bn on gpsimd

LAST_RESULT = None


def _build_nc():
    nc = bacc.Bacc(
        "TRN2",
        target_bir_lowering=False,
        debug=False,
        enable_asserts=False,
        num_devices=B,
    )

    x8_d = nc.dram_tensor("x8", (P, 2, 2, T), FP8, kind="ExternalInput")
    xb_d = nc.dram_tensor("xb", (P, 4, T), BF16, kind="ExternalInput")
    wz8_d = nc.dram_tensor("wz8", (P, 2, 2, H), FP8, kind="ExternalInput")
    wh_d = nc.dram_tensor("wh", (P, 4, H), BF16, kind="ExternalInput")
    # smalls cols: [0:4] bz, [4:8] bh, [8:12] bh+0.5, [12:16] g(h_0) carry
    smalls_d = nc.dram_tensor("smalls", (P, 16), F32, kind="ExternalInput")
    hT_d = nc.dram_tensor("hT", (H, T), EW, kind="ExternalOutput")

    AF = mybir.ActivationFunctionType
    OP = mybir.AluOpType
    DR = mybir.MatmulPerfMode.DoubleRow

    from contextlib import ExitStack

    with tile.TileContext(nc) as tc, ExitStack() as ctx:
        wpool = ctx.enter_context(tc.tile_pool(name="weights", bufs=1))
        xpool = ctx.enter_context(tc.tile_pool(name="xtiles", bufs=2))
        spool = ctx.enter_context(tc.tile_pool(name="work", bufs=3))
        hpool = ctx.enter_context(tc.tile_pool(name="hout", bufs=8))
        ppool = ctx.enter_context(tc.tile_pool(name="psum", bufs=2, space="PSUM"))

        smalls = wpool.tile([P, 16], F32, name="smalls")
        nc.gpsimd.dma_start(smalls[:], smalls_d.ap()[:])

        wz8_sb = wpool.tile([P, 2, 2, H], FP8, name="wz8_sb")
        wh_sb = wpool.tile([P, 4, H], BF16, name="wh_sb")
        nc.scalar.dma_start(wz8_sb[:], wz8_d.ap()[:])
        nc.scalar.dma_start(wh_sb[:], wh_d.ap()[:])

        def dma_x(ts0, clen):
            x8_t = xpool.tile([P, 2, 2, 1024], FP8, name="x8t", tag="x8")
            nc.sync.dma_start(x8_t[:, :, :, :clen], x8_d.ap()[:, :, :, ts0:ts0 + clen])
            xb_t = xpool.tile([P, 4, 1024], BF16, name="xbt", tag="xb")
            nc.sync.dma_start(xb_t[:, :, :clen], xb_d.ap()[:, :, ts0:ts0 + clen])
            return x8_t, xb_t

        starts = [sum(CHUNKS[:i]) for i in range(len(CHUNKS))]
        x_cur = dma_x(starts[0], CHUNKS[0])

        # PE p-state warmup on zeroed dummies while setup DMAs fly
        dwa = wpool.tile([P, 128], BF16, name="dwa")
        nc.gpsimd.memset(dwa[:], 0.0)
        dwb = wpool.tile([P, 512], BF16, name="dwb")
        nc.gpsimd.memset(dwb[:], 0.0)
        for _ in range(18):
            dp = ppool.tile([P, 512], F32, name="dp", tag="kp")
            nc.tensor.matmul(dp[:], dwa[:], dwb[:], start=True, stop=True)

        h_prev = [None] * HB

        for ci, (ts0, clen) in enumerate(zip(starts, CHUNKS)):
            x8_t, xb_t = x_cur
            if ci + 1 < len(CHUNKS):
                x_nxt = dma_x(starts[ci + 1], CHUNKS[ci + 1])

            ncc = clen // 512
            for hb in range(HB):
                hs = slice(hb * P, (hb + 1) * P)
                ms = slice(hb * P, (hb + 1) * P)

                kp = ppool.tile([P, 1024], F32, name="kp", tag="kp")
                for pair in range(2):
                    for cc in range(ncc):
                        cs = slice(cc * 512, (cc + 1) * 512)
                        nc.tensor.matmul(
                            kp[:, cs], wz8_sb[:, pair, :, ms],
                            x8_t[:, pair, :, cs],
                            start=(pair == 0), stop=(pair == 1),
                            perf_mode=DR,
                        )

                z_t = spool.tile([P, 1024], EW, name="z_t", tag="z")
                nc.scalar.activation(
                    z_t[:, :clen], kp[:, :clen], AF.Sigmoid,
                    bias=smalls[:, hb:hb + 1], scale=1.0,
                )

                wp = ppool.tile([P, 1024], F32, name="wp", tag="wp")
                for db in range(4):
                    for cc in range(ncc):
                        cs = slice(cc * 512, (cc + 1) * 512)
                        nc.tensor.matmul(
                            wp[:, cs], wh_sb[:, db, ms],
                            xb_t[:, db, cs],
                            start=(db == 0), stop=(db == 3),
                        )

                s_t = spool.tile([P, 1024], EW, name="s_t", tag="s")
                nc.scalar.activation(
                    s_t[:, :clen], wp[:, :clen], AF.Sigmoid,
                    bias=smalls[:, 4 + hb:5 + hb], scale=1.0,
                )
                t_t = spool.tile([P, 1024], EW, name="t_t", tag="t")
                nc.scalar.activation(
                    t_t[:, :clen], wp[:, :clen], AF.Identity,
                    bias=smalls[:, 8 + hb:9 + hb], scale=1.0,
                )

                a_t = spool.tile([P, 1024], EW, name="a_t", tag="a")
                nc.vector.tensor_scalar(
                    a_t[:, :clen], z_t[:, :clen], -1.0, 1.0,
                    op0=OP.mult, op1=OP.add,
                )
                g_t = spool.tile([P, 1024], EW, name="g_t", tag="g")
                nc.vector.tensor_tensor(
                    g_t[:, :clen], t_t[:, :clen], s_t[:, :clen], op=OP.max,
                )
                bn_t = spool.tile([P, 1024], EW, name="bn_t", tag="bn")
                bn_eng = nc.gpsimd if BN_GP else nc.vector
                bn_eng.tensor_tensor(
                    bn_t[:, :clen], z_t[:, :clen], g_t[:, :clen], op=OP.mult,
                )

                h_t = hpool.tile([P, 1024], EW, name="h_t", tag="h")
                last_tile = (ci == len(CHUNKS) - 1) and (hb == HB - 1)
                nscan = 2 if last_tile else 1
                ssub = clen // nscan
                for u in range(nscan):
                    us = slice(u * ssub, (u + 1) * ssub)
                    if u > 0:
                        init = h_t[:, u * ssub - 1:u * ssub]
                    elif ci == 0:
                        init = smalls[:, 12 + hb:13 + hb]
                    else:
                        pt, plen = h_prev[hb]
                        init = pt[:, plen - 1:plen]
                    nc.vector.tensor_tensor_scan(
                        h_t[:, us], a_t[:, us], bn_t[:, us], init,
                        op0=OP.mult, op1=OP.add,
                    )
                    nc.sync.dma_start(
                        hT_d.ap()[hs, ts0 + u * ssub:ts0 + (u + 1) * ssub],
                        h_t[:, us],
                    )
                h_prev[hb] = (h_t, clen)

            if ci + 1 < len(CHUNKS):
                x_cur = x_nxt

    nc.compile()
    return nc


def _host_prep(x, h_0, Wz, bz, Wh, bh):
    x = np.asarray(x, dtype=np.float32)
    h_0 = np.asarray(h_0, dtype=np.float32)
    Wz = np.asarray(Wz, dtype=np.float32)
    bz = np.asarray(bz, dtype=np.float32)
    Wh = np.asarray(Wh, dtype=np.float32)
    bh = np.asarray(bh, dtype=np.float32)

    bf = mybir.dt.np(BF16)
    f8 = mybir.dt.np(FP8)

    xT = np.transpose(x, (0, 2, 1))                       # (B, D, T)
    xb = np.ascontiguousarray(
        xT.reshape(B, 4, P, T).transpose(0, 2, 1, 3)).astype(bf)   # (B,P,4,T)
    x8 = np.ascontiguousarray(
        xT.reshape(B, 2, 2, P, T).transpose(0, 3, 1, 2, 4)).astype(f8)  # (B,P,2,2,T)

    WzT = Wz.T                                             # (D, H)
    wz8 = np.ascontiguousarray(
        WzT.reshape(2, 2, P, H).transpose(2, 0, 1, 3)).astype(f8)  # (P,2,2,H)
    WhT = Wh.T
    wh = np.ascontiguousarray(
        WhT.reshape(4, P, H).transpose(1, 0, 2)).astype(bf)        # (P,4,H)

    sig = 1.0 / (1.0 + np.exp(-h_0.astype(np.float64)))
    h0g = (np.minimum(sig, 0.5) + np.maximum(h_0, 0.0)).astype(np.float32)

    smalls = np.zeros((B, P, 16), dtype=np.float32)
    for hb in range(HB):
        blk = slice(hb * P, (hb + 1) * P)
        smalls[:, :, hb] = bz[blk]
        smalls[:, :, 4 + hb] = bh[blk]
        smalls[:, :, 8 + hb] = bh[blk] + 0.5
        smalls[:, :, 12 + hb] = h0g[:, blk]
    smalls = np.ascontiguousarray(smalls)

    in_maps = []
    for i in range(B):
        in_maps.append({
            "x8": x8[i],
            "xb": xb[i],
            "wz8": wz8,
            "wh": wh,
            "smalls": smalls[i],
        })
    return in_maps


def kernel(x, h_0, Wz, bz, Wh, bh):
    global LAST_RESULT
    in_maps = _host_prep(x, h_0, Wz, bz, Wh, bh)
    nc = _build_nc()
    res = run_bass_kernel_spmd(
        nc,
        in_maps,
        core_ids=list(range(B)),
        trace=bool(int(os.environ.get("MINGRU_TRACE", "0"))),
    )
    LAST_RESULT = res
    out = np.empty((B, T, H), dtype=np.float32)
    for i in range(B):
        out[i] = np.asarray(res.results[i]["hT"]).astype(np.float32).T
    return out
